# revision 37
# baseline (speedup 1.0000x reference)
"""Trainium2 Bass kernel for nn_CompressedInteractionNetwork_9105330667837.

Algorithm: the network output is (B,1) only, so the 3-layer CIN collapses
algebraically to a per-(b,d)-column quartic form evaluated as
    out[b] = B_const + sum_d [ g(x).t(x) + x.u(x) ],   x = x0[b,:,d] in R^32
with g[o] = x^T W1[o] x (64 quadratic forms), t[k] = x^T U3[k] x + V2[k].x,
u = Asym x + s23.  All quadratic forms go through a shared squares basis:
z = LIN @ x (480 pair-sums + 32 identity rows), basis = [z^2 ; x ; x_m x_{m+16}],
then [g;t] = R @ basis.

v3 layout (engine-op cost == free-dim size, so minimize op count/width):
 - 4 LIN matmuls (K=32, M=128, N=512) packed into the PE's four 32-row
   strips via tile_position -> concurrent.  x is DMA-replicated into all
   four partition quadrants (no on-chip copies).
 - chain 3 carries 96 pair-squares + 32 x^2 (identity rows squared); the
   Asym matvec is a separate small matmul (K=32,M=32) row-tiled at strip 2.
 - big contraction: 4x K=128 + 1x K=48 ([x ; x_m x_{m+16}]) accumulating
   matmuls.
 - products read both PSUM halves directly (no gs copy); one merged
   [96,512] product/stt tile; ONE segmented tensor_reduce per chunk into
   a [96,256] accumulator; ONE final K=96 matmul.

Sharding: data-parallel over batch across 8 cores (weights replicated).
"""

import numpy as np
from contextlib import ExitStack

import concourse.bass as bass
from concourse import bacc
import concourse.mybir as mybir
import concourse.tile as tile
from concourse.bass_utils import run_bass_kernel_spmd
from concourse import dve_ops as _dvo
from concourse.dve_spec import Spec as _Spec, Src0 as _Src0, Bin as _Bin, AluOp as _AluOp


def _register_square_op():
    if "SQUARE_ANT" in _dvo._SUB_OPCODE_FOR_NAME:
        return _dvo.CUSTOM_DVE_SPECS and [op for op in _dvo.OPS if op.name == "SQUARE_ANT"][0]
    op = _dvo.DveOp(
        "SQUARE_ANT",
        _Spec(
            body=_Bin(_AluOp.MULTIPLY, _Src0, _Src0),
            reference=lambda in0, in1, s0, s1, imm2: (
                in0.astype(np.float32) * in0.astype(np.float32)
            ),
        ),
        subdim=False,
        uops_sha={},
    )
    _dvo.OPS.append(op)
    _dvo.CUSTOM_DVE_SPECS[op.name] = op.spec
    _dvo._SUB_OPCODE_FOR_NAME[op.name] = max(_dvo._SUB_OPCODE_FOR_NAME.values()) + 1
    for ver in ("v3", "v4"):
        try:
            op.compile(ver)
        except ValueError as e:
            import re as _re
            m = _re.search(r": ([0-9a-f]{16}) ", str(e))
            if m is None:
                raise
            op.uops_sha[ver] = m.group(1)
            _dvo._COMPILE_CACHE.pop((op.name, ver), None)
            op.compile(ver)
    return op


SQUARE_ANT = _register_square_op()

B, F, D = 2048, 32, 64
NCORES = 8
BC = B // NCORES            # 256 batches per core
CHUNK_B = 8                 # batches per chunk
CP = CHUNK_B * D            # 512 columns per chunk
NCH = BC // CHUNK_B         # 32
GROUP = 4                   # chunks per DMA group
NG = NCH // GROUP           # 8

SPECIAL = [(m, m + 16) for m in range(16)]          # pairs done as direct products
_SP = set(SPECIAL)
PAIRS = [(a, b) for a in range(F) for b in range(a + 1, F) if (a, b) not in _SP]
assert len(PAIRS) == 480

f32 = mybir.dt.float32
f32r = mybir.dt.float32r
bf16 = mybir.dt.bfloat16


def fold_weights(W1, b1, W2, b2, W3, b3, W_out, b_out):
    """Host-side folding. Returns dict of small fp32 arrays + bconst float."""
    W1, b1, W2, b2, W3, b3, W_out, b_out = [
        np.asarray(a, dtype=np.float64) for a in (W1, b1, W2, b2, W3, b3, W_out, b_out)
    ]
    w1, w2, w3 = W_out[0:64, 0], W_out[64:128, 0], W_out[128:192, 0]

    V2 = np.einsum("o,ohm->hm", w2, W2)           # (64,32)
    V3 = np.einsum("o,ohm->hm", w3, W3)           # (64,32)
    U3 = np.einsum("hkm,hn->kmn", W2, V3)         # (64,32,32)
    V1 = np.einsum("o,ohm->hm", w1, W1)           # (32,32)
    Le = np.einsum("k,kmn->mn", b1, U3)           # (32,32)
    A = V1 + Le
    Asym = (A + A.T) / 2
    s23 = V2.T @ b1 + V3.T @ b2                   # (32,)
    bconst = D * (w1 @ b1 + w2 @ b2 + w3 @ b3) + b_out[0]

    M1s = (W1 + W1.transpose(0, 2, 1)) / 2        # 64 sym forms for g
    U3s = (U3 + U3.transpose(0, 2, 1)) / 2        # 64 sym forms for t

    # LIN lhsT: (32, 4*128). Chain j cols: j<3 -> PAIRS[128j:128j+128] sums;
    # chain 3 -> PAIRS[384:480] sums (96) + Asym rows (32, read raw by stt).
    LINW = np.zeros((F, 4 * 128))
    for j in range(4):
        rows = PAIRS[128 * j: 128 * (j + 1)]
        for i, (a, b_) in enumerate(rows):
            LINW[a, 128 * j + i] += 1.0
            LINW[b_, 128 * j + i] += 1.0
    LINW[:, 128 * 3 + 96: 128 * 3 + 128] = Asym.T  # outputs 96..127 = Asym @ x
    # Row-tiled LIN lhsT: [128, 128], chain j on partitions 32j..32j+31.
    LINW4 = LINW.reshape(F, 4, 128).transpose(1, 0, 2).reshape(128, 128)

    # Big-matmul lhsT: RW (128, 5*128).
    # outputs: m<64 -> form M1s[m], lin v=0 ; m>=64 -> form U3s[m-64], v=V2[m-64]
    forms = np.concatenate([M1s, U3s], axis=0)    # (128, 32, 32)
    linv = np.concatenate([np.zeros((64, F)), V2], axis=0)  # (128, 32)

    RW = np.zeros((128, 5 * 128))
    for j in range(4):
        rows = PAIRS[128 * j: 128 * (j + 1)]
        for i, (a, b_) in enumerate(rows):
            RW[i, 128 * j:128 * (j + 1)] = forms[:, a, b_]
    # chain 4 (K=80): rows 0-31 x^2; 32-63 x (linear); 64-79 x_m x_{m+16}
    corr = np.zeros((128, F))
    for (a, b_) in PAIRS:
        corr[:, a] += forms[:, a, b_]
        corr[:, b_] += forms[:, a, b_]
    for m in range(F):
        RW[m, 128 * 4:128 * 5] = forms[:, m, m] - corr[:, m]
        RW[32 + m, 128 * 4:128 * 5] = linv[:, m]
    for i, (a, b_) in enumerate(SPECIAL):
        RW[64 + i, 128 * 4:128 * 5] = 2.0 * forms[:, a, b_]

    return {
        "linw4": LINW4.astype(np.float32),
        "rw": RW.astype(np.float32),
        "s23": s23.reshape(F, 1).astype(np.float32),
        "ones": np.ones((96, 1), dtype=np.float32),
    }, float(bconst)


_module_cache = {}


CFG = {
    "sq_scalar_cols": 1280,   # ScalarE share of chain0-2 squares
    "sq2_eng": "scalar",      # chain-3 (96-row) square engine
    "xy_eng": "gp",           # "vec" | "gp"
    "x2_eng": "gp",           # x^2 producer
    "bf16": True,             # bf16 matmul pipeline (enables FWL weight loads)
    "warmup_mms": 16,         # dense PE warm-up matmuls to trigger HAM 8/8
    "big_bufs": 3,
    "xg_bufs": 3,
    "chn_bufs": 2,
}


def build_module(bconst: float):
    key = (round(bconst, 12), tuple(sorted(CFG.items())))
    if key in _module_cache:
        return _module_cache[key]
    nc = bacc.Bacc("TRN2", target_bir_lowering=False)
    MD = bf16 if CFG["bf16"] else f32r
    # x stored host-transposed [F, BC, D] so SBUF loads are contiguous
    x_d = nc.dram_tensor("x", [F, BC, D], MD, kind="ExternalInput")
    linw4_d = nc.dram_tensor("linw4", [128, 128], MD, kind="ExternalInput")
    rw_d = nc.dram_tensor("rw", [128, 5 * 128], MD, kind="ExternalInput")
    s23_d = nc.dram_tensor("s23", [F, 1], f32, kind="ExternalInput")
    ones_d = nc.dram_tensor("ones", [96, 1], f32r, kind="ExternalInput")
    out_d = nc.dram_tensor("out", [1, BC], f32, kind="ExternalOutput")

    SQ = mybir.ActivationFunctionType.Square
    CP_ACT = mybir.ActivationFunctionType.Copy
    ADD = mybir.AluOpType.add
    MULT = mybir.AluOpType.mult
    AXX = mybir.AxisListType.X

    with tile.TileContext(nc) as tc, ExitStack() as ctx:
        const = ctx.enter_context(tc.tile_pool(name="const", bufs=1))
        xp = ctx.enter_context(tc.tile_pool(name="xp", bufs=CFG["xg_bufs"]))
        xsp = ctx.enter_context(tc.tile_pool(name="xsp", bufs=CFG["xg_bufs"]))
        ch4p = ctx.enter_context(tc.tile_pool(name="ch4p", bufs=CFG["xg_bufs"]))
        chp = ctx.enter_context(tc.tile_pool(name="chp", bufs=CFG["chn_bufs"]))
        gsp = ctx.enter_context(tc.tile_pool(name="gsp", bufs=2))
        prp = ctx.enter_context(tc.tile_pool(name="prp", bufs=2))
        prap = ctx.enter_context(tc.tile_pool(name="prap", bufs=1))
        outp = ctx.enter_context(tc.tile_pool(name="outp", bufs=1))
        linps = ctx.enter_context(tc.tile_pool(name="linps", bufs=1, space="PSUM"))
        bigps = ctx.enter_context(
            tc.tile_pool(name="bigps", bufs=CFG["big_bufs"], space="PSUM"))

        linw4_t = const.tile([128, 128], MD)
        nc.sync.dma_start(linw4_t[:], linw4_d[:])
        rw_t = const.tile([128, 5 * 128], MD)
        nc.sync.dma_start(rw_t[:], rw_d[:])
        s23_t = const.tile([F, 1], f32)
        nc.sync.dma_start(s23_t[:], s23_d[:])
        ones_t = const.tile([96, 1], f32r)
        nc.sync.dma_start(ones_t[:], ones_d[:])

        pracc = prap.tile([96, BC], f32r)

        # HAM warm-up: ~16 dense matmuls (~5us) push the PE clock-gate to
        # 8/8 (2.4 GHz); steady-state gaps stay < the ~3.4us MID window so
        # it never re-throttles.  Uses rw_t as both operands; results unused.
        for w in range(CFG["warmup_mms"]):
            wp = bigps.tile([128, CP], f32, tag="bp", name="warm")
            nc.tensor.matmul(wp[:], rw_t[0:128, 0:128], rw_t[0:128, 0:512],
                             start=True, stop=True)

        SCC = CFG["sq_scalar_cols"]

        def emit_lin(c):
            """DMA (at group heads) + LIN matmuls + xy for chunk c."""
            g, ci = divmod(c, GROUP)
            if ci == 0:
                b0 = g * GROUP * CHUNK_B
                nb = GROUP * CHUNK_B
                xsrc = x_d[:, b0:b0 + nb, :]              # (32, nb, 64)
                xg_t = xp.tile([128, GROUP * CP], MD, tag="x")
                for q in range(4):
                    nc.sync.dma_start(
                        xg_t[32 * q:32 * (q + 1)].rearrange(
                            "k (b d) -> k b d", b=nb),
                        xsrc,
                    )
                ch4g = ch4p.tile([80, GROUP * CP], MD, tag="ch4")
                nc.sync.dma_start(
                    ch4g[32:64].rearrange("k (b d) -> k b d", b=nb), xsrc
                )
                xs_g = xsp.tile([16, GROUP * CP], MD, tag="xs")
                nc.sync.dma_start(
                    xs_g[:].rearrange("k (b d) -> k b d", b=nb),
                    x_d[16:32, b0:b0 + nb, :],
                )
                emit_lin.group = (xg_t, ch4g, xs_g)
            xg_t, ch4g, xs_g = emit_lin.group
            cs = slice(ci * CP, (ci + 1) * CP)
            x_t = xg_t[0:32, cs]
            ch4 = ch4g[:, cs]
            xs_t = xs_g[:, cs]

            # 4 concurrent row-tiled LIN matmuls (K=32 each); high priority
            # so the greedy scheduler runs them before the pending big(c-1)
            # the moment lp frees (else the pipeline collapses serial).
            lp = linps.tile([128, 4 * CP], f32, tag="lp")
            with tc.high_priority():
                for q in range(4):
                    nc.tensor.matmul(
                        lp[:, q * CP:(q + 1) * CP],
                        linw4_t[32 * q:32 * (q + 1), :],
                        xg_t[32 * q:32 * (q + 1), cs],
                        start=True, stop=True, tile_position=(32 * q, 0),
                    )

            # chain 4 extras: x^2 and x_m * x_{m+16}
            if CFG["x2_eng"] == "gp":
                nc.gpsimd.tensor_mul(ch4[0:32], x_t, x_t)
            else:
                nc.scalar.activation(ch4[0:32], x_t, SQ)
            if CFG["xy_eng"] == "gp":
                nc.gpsimd.tensor_mul(ch4[64:80], x_t[0:16], xs_t)
            else:
                nc.vector.tensor_mul(ch4[64:80], x_t[0:16], xs_t)
            return (c, xg_t, cs, x_t, ch4, lp, None)

        def emit_sq(st):
            """Squares evacuating lp for chunk c; ScalarE + DVE split.
            Chain-3 cols [1536,2048) square only rows 0:96 (Asym rows raw)."""
            c, xg_t, cs, x_t, ch4, lp, _ = st
            chn = chp.tile([128, 4 * CP], MD, tag="ch")
            # high priority: lp release gates the next LIN; do not let the
            # scheduler slot gs/prod ahead of these.
            with tc.high_priority():
                if CFG["sq2_eng"] == "scalar":
                    nc.scalar.activation(
                        chn[0:96, 3 * CP:4 * CP], lp[0:96, 3 * CP:4 * CP], SQ)
                else:
                    nc.vector._custom_dve(
                        SQUARE_ANT, out=chn[0:96, 3 * CP:4 * CP],
                        in0=lp[0:96, 3 * CP:4 * CP])
                s1 = min(SCC, 3 * CP)
                if s1 > 0:
                    nc.scalar.activation(chn[:, 0:s1], lp[:, 0:s1], SQ)
                if s1 < 3 * CP:
                    nc.vector._custom_dve(
                        SQUARE_ANT, out=chn[:, s1:3 * CP], in0=lp[:, s1:3 * CP])
            return (c, xg_t, cs, x_t, ch4, lp, chn)

        def emit_back(st):
            """Big contraction + products + reduction for chunk c."""
            c, xg_t, cs, x_t, ch4, lp, chn = st
            # big contraction: 5 accumulating matmuls
            bp = bigps.tile([128, CP], f32, tag="bp")
            for j in range(3):
                nc.tensor.matmul(
                    bp[:], rw_t[0:128, 128 * j:128 * (j + 1)],
                    chn[:, j * CP:(j + 1) * CP],
                    start=(j == 0), stop=False,
                )
            nc.tensor.matmul(
                bp[:], rw_t[0:96, 384:512], chn[0:96, 3 * CP:4 * CP],
                start=False, stop=False,
            )
            nc.tensor.matmul(
                bp[:], rw_t[0:80, 512:640], ch4[0:80],
                start=False, stop=True,
            )

            # stt first on DVE (reads Asym rows raw from lin PSUM)
            pr96 = prp.tile([96, CP], f32, tag="pr96")
            with tc.high_priority():
                nc.vector.scalar_tensor_tensor(
                    pr96[64:96], lp[96:128, 3 * CP:4 * CP], s23_t[:], x_t,
                    ADD, MULT
                )
            # evacuate the g-half on ScalarE; DVE multiplies vs the PSUM t-half
            # (SBUF x SBUF ops require equal base partitions — NCC_IBIR297)
            gs = gsp.tile([64, CP], f32, tag="gs")
            nc.scalar.activation(gs[:], bp[0:64], CP_ACT)
            nc.vector.tensor_mul(pr96[0:64], gs[:], bp[64:128])
            with nc.allow_low_precision(reason="f32r accumulator is fp32 bits"):
                nc.vector.tensor_reduce(
                    pracc[:, c * CHUNK_B:(c + 1) * CHUNK_B],
                    pr96[:].rearrange("p (b d) -> p b d", b=CHUNK_B),
                    AXX, ADD,
                )

        # software pipeline: squares(i) issue right after LIN(i) so lp is
        # released promptly; big(i-1) fills the PE while they evacuate.
        prev = None
        for c in range(NCH):
            st = emit_sq(emit_lin(c))
            if prev is not None:
                emit_back(prev)
            prev = st
        emit_back(prev)

        # final: one K=96 matmul over the accumulated per-batch sums
        # (writes into a bp-pool buffer; no dedicated PSUM bank needed)
        fin_bp = bigps.tile([128, CP], f32, tag="bp", name="fin_bp")
        fp = fin_bp[0:1, 0:BC]
        nc.tensor.matmul(fp, ones_t[:], pracc[:], start=True, stop=True)
        out_sb = outp.tile([1, BC], f32)
        nc.scalar.activation(out_sb[:], fp, CP_ACT, bias=float(bconst))
        nc.sync.dma_start(out_d[:], out_sb[:])

    nc.compile()
    _module_cache[key] = nc
    return nc


def _run(inputs, trace=False, **kw):
    folded, bconst = fold_weights(
        inputs["W1"], inputs["b1"], inputs["W2"], inputs["b2"],
        inputs["W3"], inputs["b3"], inputs["W_out"], inputs["b_out"],
    )
    nc = build_module(bconst)
    x0 = np.ascontiguousarray(np.asarray(inputs["x0"], dtype=np.float32))
    if CFG["bf16"]:
        import ml_dtypes
        md = ml_dtypes.bfloat16
        folded = dict(folded)
        for k in ("linw4", "rw"):
            folded[k] = folded[k].astype(md)
        x0 = x0.astype(md)
    in_maps = []
    for c in range(NCORES):
        m = dict(folded)
        # host-transpose to [F, BC, D] so device DMA reads are contiguous
        m["x"] = np.ascontiguousarray(x0[BC * c:BC * (c + 1)].transpose(1, 0, 2))
        in_maps.append(m)
    res = run_bass_kernel_spmd(nc, in_maps, core_ids=list(range(NCORES)),
                               trace=trace, **kw)
    out = np.concatenate(
        [res.results[c]["out"].reshape(BC, 1) for c in range(NCORES)], axis=0
    )
    return out, res


def kernel(**inputs) -> np.ndarray:
    out, _ = _run(inputs, trace=False)
    return out
